# revision 18
# baseline (speedup 1.0000x reference)
"""MoE location-expert router kernel for Trainium2 (8 NeuronCores).

Problem: out[i] = W[ptr[i] % 8] @ x[i] + b[ptr[i] % 8]
  x  [4096, 1024] f32, W [8, 32000, 1024] f32, b [8, 32000] f32 (zeros)
  out [4096, 32000] f32

Strategy (vocab / tensor-parallel sharding):
  - Each of the 8 cores owns a 4000-wide slice of the vocab dim of ALL
    8 experts -> identical SPMD program on every core, perfectly load
    balanced regardless of the routing distribution.
  - Host routes tokens: sort by expert, pad each expert group to a
    multiple of 32 (PE sub-array granularity). Global 128-token tiles
    may span two experts; such boundary tiles are issued as 32/64-wide
    column-group strips (tile_position) whose matmul chains execute
    CONCURRENTLY in disjoint PE column groups -> almost no padding
    waste (vs. padding each expert to 128).
  - Loop nest: tile-major; per (tile, vocab-bank): kc-contiguous
    accumulation chains into one PSUM bank (bank switches only between
    chains; HW-measured ~44ns/MM penalty for per-MM bank cycling).
  - Whole expert weight slice [1024, 4000] fp16 resident in SBUF,
    ~2 experts live, prefetched one expert ahead.
  - PSUM->SBUF drain copies alternate between DVE and ACT engines.
  - Host scatters the 8 x [4096, 4000] results back to original token
    order / full vocab.
"""

import contextlib
import os

import numpy as np

import concourse.bacc as bacc
import concourse.bass as bass
import concourse.mybir as mybir
import concourse.tile as tile
from concourse.bass_utils import run_bass_kernel_spmd

E = 8          # experts
D = 1024       # d_model
V = 32000      # vocab
B = 4096       # tokens
NCORES = 8
VS = V // NCORES       # vocab slice per core (4000)
KT = 128               # contraction tile (partition dim)
KC = D // KT           # 8 K-chunks
MT = 128               # token tile (PSUM partition dim)
PAD = 32               # per-expert token padding granularity
NT = 500               # vocab tile (moving free dim, <=512 for one PSUM bank)
NV = VS // NT          # 8 vocab tiles per core

MODE = os.environ.get("KERNEL_MODE", "fp16")

_program_cache = {}


def _pad32(c):
    return int(-(-c // PAD) * PAD)


def _plan(counts):
    """Token layout: per-expert pad to 32, global pad to 128.

    Returns (m_total, tiles) where tiles[t] is a list of segments
    (e, lo, hi, loc_off): tile-local columns [lo, hi) hold expert e's
    padded rows starting at expert-local row loc_off.
    """
    pe32 = [_pad32(c) for c in counts]
    starts = np.concatenate([[0], np.cumsum(pe32)]).astype(int)
    m_used = int(starts[-1])
    m_total = int(-(-m_used // MT) * MT)
    tiles = []
    for t in range(m_total // MT):
        lo_g, hi_g = t * MT, (t + 1) * MT
        segs = []
        for e in range(E):
            a = max(lo_g, int(starts[e]))
            b = min(hi_g, int(starts[e + 1]))
            if a < b:
                segs.append((e, a - lo_g, b - lo_g, a - int(starts[e])))
        tiles.append(segs)
    return m_total, tiles


def _strips(lo, hi, split=False):
    """Split tile-local col range [lo, hi) into PE-legal column strips:
    width 128 @ 0, width<=64 @ {0,64}, width<=32 @ {0,32,64,96}.

    With split=True, full 128-wide tiles are emitted as two 64-wide
    strips whose matmul chains run CONCURRENTLY in disjoint PE column
    groups; one chain's LDWEIGHTS overlaps the other's streaming, which
    hides most of the per-matmul weight-load cost."""
    out = []
    a = lo
    while a < hi:
        if a == 0 and hi == 128 and not split:
            out.append((0, 128))
            a = 128
        elif a % 64 == 0 and hi - a >= 64:
            out.append((a, a + 64))
            a += 64
        else:
            out.append((a, min(a + 32, hi)))
            a += 32
    return out


def _build_program(counts, mode, repeat=1, split=False):
    """Trace the SPMD Tile program for the given per-expert counts."""
    counts = [int(c) for c in counts]
    m_total, tiles = _plan(counts)
    val_starts = np.concatenate([[0], np.cumsum(counts)]).astype(int)

    if mode == "fp16":
        io_dt = mybir.dt.float16
    elif mode == "bf16":
        io_dt = mybir.dt.bfloat16
    else:
        io_dt = mybir.dt.float32r

    nc = bacc.Bacc("TRN2", target_bir_lowering=False, debug=False,
                   enable_asserts=False, num_devices=NCORES)

    xT = nc.dram_tensor("xT", [D, m_total], io_dt, kind="ExternalInput").ap()
    wT = nc.dram_tensor("wT", [E, D, VS], io_dt, kind="ExternalInput").ap()
    out_dt = mybir.dt.float16 if mode == "fp16" else mybir.dt.float32
    out = nc.dram_tensor("out", [B, VS], out_dt, kind="ExternalOutput").ap()

    xT_r = xT.rearrange("(kc p) m -> p kc m", p=KT)
    T = m_total // MT

    with tile.TileContext(nc) as tc:
        with (
            tc.tile_pool(name="xp", bufs=4) as xpool,
            tc.tile_pool(name="wp", bufs=20) as wpool,
            tc.tile_pool(name="op", bufs=8) as opool,
            tc.tile_pool(name="ps", bufs=NV, space="PSUM") as pspool,
            contextlib.ExitStack() as rep_ctx,
        ):
            if repeat > 1:
                rep_ctx.enter_context(tc.For_i(0, repeat))

            wk = {}     # expert -> list of KC weight tiles [128, VS]
            loaded = [-1]

            def load_expert(e):
                if e >= E or e in wk:
                    return
                wT_e = wT[e].rearrange("(kc p) v -> p kc v", p=KT)
                ts_ = []
                for v in range(NV):
                    # one [128, KC, NT] tile per vocab-bank: the first chain
                    # of an expert is gated on 1 MB, not the whole 8 MB
                    wt_t = wpool.tile([KT, KC, NT], io_dt, name=f"w{e}_{v}",
                                      tag="w")
                    # alternate between the two HWDGE queues (SP / ACT)
                    eng = nc.scalar if (v + e) % 2 == 0 else nc.sync
                    eng.dma_start(out=wt_t[:, :, :],
                                  in_=wT_e[:, :, v * NT:(v + 1) * NT])
                    ts_.append(wt_t)
                wk[e] = ts_
                loaded[0] = max(loaded[0], e)

            # first tile index that uses each expert, for prefetch pacing
            first_tile = {}
            for tt, ss in enumerate(tiles):
                for (e_, _, _, _) in ss:
                    first_tile.setdefault(e_, tt)
            present = sorted(first_tile)       # experts with any tokens
            order_pos = [0]                    # next index into `present`

            def load_x(t):
                # x tiles come in pairs of token tiles, one DMA per K-chunk
                # (keeps the first matmul gated on 128 KB, not megabytes)
                w_cols = min(2 * MT, m_total - t * MT)
                xe_t = xpool.tile([KT, KC, w_cols], io_dt, name="xe", tag="x")
                for kc in range(KC):
                    nc.sync.dma_start(
                        out=xe_t[:, kc, :],
                        in_=xT_r[:, kc, t * MT:t * MT + w_cols],
                    )
                return xe_t

            xe_tiles = {0: load_x(0)}   # x BEFORE any weights
            if present:
                load_expert(present[0])
                order_pos[0] = 1
            if T > 2:
                xe_tiles[2] = load_x(2)     # one pair ahead

            for t in range(T):
                segs = tiles[t]
                # prefetch weights ~4 tiles before an expert's first use
                while (order_pos[0] < len(present)
                       and t >= first_tile[present[order_pos[0]]] - 4):
                    load_expert(present[order_pos[0]])
                    order_pos[0] += 1
                if t % 2 == 0:
                    if t + 2 < T and (t + 2) not in xe_tiles:
                        xe_tiles[t + 2] = load_x(t + 2)
                    xe = xe_tiles.pop(t)
                xoff = (t % 2) * MT

                # strips with at least one valid (un-padded) row
                chains = []
                for (e, lo, hi, loff) in segs:
                    for (a, b) in _strips(lo, hi, split=split):
                        if loff + (a - lo) < counts[e]:
                            chains.append((e, a, b))

                for v in range(NV):
                    pt = pspool.tile([MT, NT], mybir.dt.float32,
                                     name=f"ps{v}", tag="ps")
                    # kc-contiguous chain per strip; strips interleave so
                    # their chains run concurrently in disjoint PE column
                    # groups (same PSUM bank, disjoint partitions).
                    for kc in range(KC):
                        for (e, a, b) in chains:
                            nc.tensor.matmul(
                                pt[a:b, :],
                                xe[:, kc, xoff + a:xoff + b],
                                wk[e][v][:, kc, :],
                                start=(kc == 0), stop=(kc == KC - 1),
                                tile_position=(0, a),
                            )
                    ot = opool.tile([MT, NT], out_dt, name="ot", tag="o")
                    if v % 2 == 0:
                        nc.vector.tensor_copy(ot[:, :], pt[:, :])
                    else:
                        nc.scalar.copy(ot[:, :], pt[:, :])
                    for (e, lo, hi, loff) in segs:
                        valid = min(hi - lo, int(counts[e]) - loff)
                        if valid <= 0:
                            continue
                        row0 = int(val_starts[e]) + loff
                        nc.sync.dma_start(
                            out=out[row0:row0 + valid,
                                    v * NT:(v + 1) * NT],
                            in_=ot[lo:lo + valid, :],
                        )
    nc.compile()
    return nc, m_total


def _get_program(counts, mode, repeat=1, split=False):
    key = (tuple(int(c) for c in counts), mode, repeat, split)
    if key not in _program_cache:
        _program_cache[key] = _build_program(counts, mode, repeat=repeat,
                                             split=split)
    return _program_cache[key]


def _prepare(x, pointer_addresses, W, mode, repeat=1, split=False):
    idx = (np.asarray(pointer_addresses).astype(np.int64) % E).astype(np.int32)
    counts = np.bincount(idx, minlength=E)
    order = np.argsort(idx, kind="stable")
    nc, m_total = _get_program(tuple(counts), mode, repeat=repeat, split=split)

    np_dt = np.dtype("float32")
    if mode == "fp16":
        np_dt = np.dtype(np.float16)
    elif mode == "bf16":
        import ml_dtypes
        np_dt = np.dtype(ml_dtypes.bfloat16)

    x = np.asarray(x, dtype=np.float32)
    xs = x[order]                      # [B, D] sorted by expert
    x_pad = np.zeros((m_total, D), dtype=np_dt)
    row = 0
    srow = 0
    for e in range(E):
        c = int(counts[e])
        x_pad[row:row + c] = xs[srow:srow + c]
        row += _pad32(c)
        srow += c
    xT = np.ascontiguousarray(x_pad.T)  # [D, m_total]

    W = np.asarray(W)
    wts = []
    for c in range(NCORES):
        Wc = W[:, c * VS:(c + 1) * VS, :]                 # [E, VS, D] view
        WTc = np.ascontiguousarray(Wc.transpose(0, 2, 1))  # [E, D, VS]
        if mode in ("fp16", "bf16"):
            WTc = WTc.astype(np_dt)
        wts.append(WTc)
    return idx, order, nc, xT, wts


def _run(x, pointer_addresses, W, b, trace=False, mode=None):
    mode = mode or MODE
    idx, order, nc, xT, wts = _prepare(x, pointer_addresses, W, mode)
    in_maps = [{"xT": xT, "wT": wts[c]} for c in range(NCORES)]
    kw = {}
    if trace:
        kw = dict(trace=True, trace_cores=[0])
    res = run_bass_kernel_spmd(nc, in_maps, list(range(NCORES)), **kw)

    out = np.empty((B, V), dtype=np.float32)
    for c in range(NCORES):
        out[order, c * VS:(c + 1) * VS] = res.results[c]["out"]

    b = np.asarray(b)
    if b.any():
        for e in range(E):
            out[idx == e] += b[e].astype(np.float32)
    return out, res


def kernel(x, pointer_addresses, W, b):
    out, _ = _run(x, pointer_addresses, W, b, trace=False)
    return out
